# revision 3
# baseline (speedup 1.0000x reference)
"""Bilateral grid slicing kernel v3 for Trainium2 (8 NeuronCores, SPMD).

Key ideas vs baseline:
- Tables are corner-shifted, cell-major, fp16-PAIR-packed: one int32 word
  holds (lo=G[ch,cell], hi=G[ch,cell+1]-G[ch,cell]) as 2 fp16. One gather
  (d=3 words) per pixel-role delivers 3 channels x both x-cells; the x-lerp
  becomes lo + wx*hi (no separate x-corner gathers).
- 16 roles per gather group: (dz,dy) corner x channel-quarter q, channel
  ch = 4*w + q  (w = word = OUTPUT channel i, q = INPUT channel j'), so the
  final affine out_i = sum_j' A[i*4+j'] * c_j' folds into the blend weights
  V16 = zy(dzdy) (x) cq(q),  cq = (r, g, b, 1).
- ONE ap_gather per tile (idx = base cell, shared by all roles) instead of 8.
- PE transposes run in fp16 (1 cycle/row) into fp16 PSUM; DVE blend runs in
  fp16 at 2x; out = sum over 16 roles of V16 * (lo + wx*hi), done as
  mult + binary tree, then the final x-lerp combines lo/hi trees.
- floor() via two-step magic-number rounding (no int roundtrip).

Distribution: core k = (view k//2, H-half k%2), as baseline.
"""
import sys

sys.path.insert(0, "/opt/trn_rl_repo")
import numpy as np

import concourse.bass as bass
import concourse.bacc as bacc
import concourse.tile as tile
from concourse import mybir
from concourse import bass_utils

F32 = mybir.dt.float32
F16 = mybir.dt.float16
I16 = mybir.dt.int16
I32 = mybir.dt.int32
Alu = mybir.AluOpType
ActFn = mybir.ActivationFunctionType

N_CORES = 8
H, W = 1080, 1920
HH = H // 2
P_CORE = HH * W                 # 1,036,800
T = 128
N_TILE = 128 * T                # 16384
N_TILES = (P_CORE + N_TILE - 1) // N_TILE   # 64
P_PAD = N_TILES * N_TILE

GL, GH, GW = 8, 16, 16
NCELL = GL * GH * GW            # 2048
NE = 2080                       # padded table entries (max idx 1791 + 273)
MAGIC = 12582912.0              # 1.5 * 2^23
HB = -0.49999997                # half bias for floor()

_cache = {}


def _build(n_tiles):
    nc = bacc.Bacc("TRN2", target_bir_lowering=False)
    n_pix = n_tiles * N_TILE
    with tile.TileContext(nc) as tc:
        with tc.tile_pool(name="dram", bufs=1, space="DRAM") as dram:
            gxy = dram.tile([n_pix, 2], F32, kind="ExternalInput", name="gxy", uniquify=False)
            rgb = dram.tile([n_pix, 3], F32, kind="ExternalInput", name="rgb", uniquify=False)
            tab = dram.tile([128, NE, 3], I32, kind="ExternalInput", name="tab", uniquify=False)
            ident = dram.tile([128, 128], F32, kind="ExternalInput", name="ident", uniquify=False)
            out = dram.tile([n_pix, 3], F32, kind="ExternalOutput", name="out", uniquify=False)
            _body(nc, tc, n_tiles, gxy, rgb, tab, ident, out)
    nc.compile()
    return nc


def _ap(t, extra_dims, offset=0):
    a = t[:] if not isinstance(t, bass.AP) else t
    return bass.AP(tensor=a.tensor, offset=a.offset + offset,
                   ap=[list(a.ap[0])] + [list(d) for d in extra_dims])


def _body(nc, tc, n_tiles, gxy, rgb, tab, ident, out):
    import contextlib
    ctx = contextlib.ExitStack()
    const = ctx.enter_context(tc.tile_pool(name="const", bufs=1))
    io = ctx.enter_context(tc.tile_pool(name="io", bufs=3))
    wk = ctx.enter_context(tc.tile_pool(name="wk", bufs=2))
    gkp = ctx.enter_context(tc.tile_pool(name="gkp", bufs=2))
    blp = ctx.enter_context(tc.tile_pool(name="blp", bufs=1))
    outp = ctx.enter_context(tc.tile_pool(name="outp", bufs=3))
    ps_idx = ctx.enter_context(tc.tile_pool(name="ps_idx", bufs=2, space="PSUM"))
    ps_x2 = ctx.enter_context(tc.tile_pool(name="ps_x2", bufs=2, space="PSUM"))

    # --- one-time setup ---------------------------------------------------
    tab_sb = const.tile([128, NE, 3], I32)
    nc.sync.dma_start(out=tab_sb[:], in_=tab[:])
    ident16 = const.tile([128, 128], F16)
    idf = const.tile([128, 128], F32)
    nc.sync.dma_start(out=idf[:], in_=ident[:])
    nc.scalar.copy(out=ident16[:], in_=idf[:])

    tab16 = tab_sb[:].bitcast(F16)      # [128, NE, 3, 2]
    tpitch = list(tab16.ap[0])          # partition pitch in f16 units

    for it in range(n_tiles):
        j0 = it * N_TILE
        gxy_t = io.tile([128, T, 2], F32, tag="gxy_t")
        nc.sync.dma_start(out=gxy_t[:], in_=gxy[j0:j0 + N_TILE, :].rearrange("(p t) c -> p t c", p=128))
        rgb_t = io.tile([128, T, 3], F32, tag="rgb_t")
        nc.sync.dma_start(out=rgb_t[:], in_=rgb[j0:j0 + N_TILE, :].rearrange("(p t) c -> p t c", p=128))

        # --- prep: coords, floors (magic), fracs, idx, zy ----------------
        ixyh = wk.tile([128, T, 2], F32, tag="ixyh")      # gxy*15 + HB
        nc.scalar.activation(ixyh[:], gxy_t[:], ActFn.Copy, scale=15.0, bias=HB)
        ixym = wk.tile([128, T, 2], F32, tag="ixym")
        nc.scalar.activation(ixym[:], ixyh[:], ActFn.Copy, bias=MAGIC)
        xy0 = wk.tile([128, T, 2], F32, tag="xy0")
        nc.scalar.activation(xy0[:], ixym[:], ActFn.Copy, bias=-MAGIC)
        # wxy = ixy - xy0 = (ixyh - HB) - xy0
        wxy = wk.tile([128, T, 2], F32, tag="wxy")
        nc.vector.scalar_tensor_tensor(wxy[:], ixyh[:], -HB, xy0[:], Alu.add, Alu.subtract)
        wx16 = wk.tile([128, T], F16, tag="wx16")
        nc.scalar.copy(out=wx16[:], in_=wxy[:, :, 0])

        iz = wk.tile([128, T], F32, tag="iz")
        nc.scalar.activation(iz[:], rgb_t[:, :, 0], ActFn.Copy, scale=0.299 * 7.0)
        nc.vector.scalar_tensor_tensor(iz[:], rgb_t[:, :, 1], 0.587 * 7.0, iz[:], Alu.mult, Alu.add)
        nc.vector.scalar_tensor_tensor(iz[:], rgb_t[:, :, 2], 0.114 * 7.0, iz[:], Alu.mult, Alu.add)
        izm = wk.tile([128, T], F32, tag="izm")
        nc.vector.tensor_scalar(izm[:], iz[:], 6.9999995, HB, Alu.min, Alu.add)
        izm2 = wk.tile([128, T], F32, tag="izm2")
        nc.scalar.activation(izm2[:], izm[:], ActFn.Copy, bias=MAGIC)
        z0 = wk.tile([128, T], F32, tag="z0")
        nc.scalar.activation(z0[:], izm2[:], ActFn.Copy, bias=-MAGIC)

        # wzs = (1-wz, wz), wys = (1-wy, wy)
        wzs = wk.tile([128, T, 2], F32, tag="wzs")
        nc.vector.tensor_tensor(wzs[:, :, 1], iz[:], z0[:], Alu.subtract)
        nc.vector.scalar_tensor_tensor(wzs[:, :, 0], z0[:], 1.0, iz[:], Alu.add, Alu.subtract)
        wys = wk.tile([128, T, 2], F32, tag="wys")
        nc.scalar.copy(out=wys[:, :, 1], in_=wxy[:, :, 1])
        # 1-wy = (y0 + 1 + HB) - ixyh_y
        nc.vector.scalar_tensor_tensor(wys[:, :, 0], xy0[:, :, 1], 1.0 + HB, ixyh[:, :, 1], Alu.add, Alu.subtract)

        # idxf = (z0*16 + y0)*16 + x0
        idxf = wk.tile([128, T], F32, tag="idxf")
        nc.vector.scalar_tensor_tensor(idxf[:], z0[:], 16.0, xy0[:, :, 1], Alu.mult, Alu.add)
        nc.vector.scalar_tensor_tensor(idxf[:], idxf[:], 16.0, xy0[:, :, 0], Alu.mult, Alu.add)

        # zy[p,t,4] = wzs (x) wys   (dzdy = dz*2+dy)
        zy = wk.tile([128, T, 4], F32, tag="zy")
        nc.vector.tensor_tensor(
            _ap(zy, [[4, T], [2, 2], [1, 2]]),
            _ap(wzs, [[2, T], [1, 2], [0, 2]]),
            _ap(wys, [[2, T], [0, 2], [1, 2]]),
            Alu.mult)

        # V16[p,t,q,dzdy] = cq * zy, cq = (r,g,b,1); rgb4 = rgb bcast over dzdy
        rgb4 = wk.tile([128, T, 3, 4], F16, tag="rgb4")
        nc.scalar.copy(out=rgb4[:], in_=_ap(rgb_t, [[3, T], [1, 3], [0, 4]]))
        v16 = wk.tile([128, T, 4, 4], F16, tag="v16")
        nc.vector.tensor_tensor(
            _ap(v16, [[16, T], [4, 3], [1, 4]]),
            _ap(zy, [[4, T], [0, 3], [1, 4]]),
            _ap(rgb4, [[12, T], [4, 3], [1, 4]]),
            Alu.mult)
        nc.scalar.copy(out=_ap(v16, [[16, T], [1, 4]], offset=12),
                       in_=_ap(zy, [[4, T], [1, 4]]))

        # --- idx: transpose -> wrapped int16 -----------------------------
        pidx = ps_idx.tile([128, 128], F32, tag="pidx")
        nc.tensor.transpose(pidx[:], idxf[:], idf[:])
        wr = wk.tile([128, 128], I16, tag="wr")
        nc.scalar.copy(out=wr[:], in_=pidx[:])

        # --- ONE gather: gk[P, i, w] = tab[P][idx_g(P)[i]] ---------------
        gk = gkp.tile([128, N_TILE // 8, 3], I32, tag="gk")
        nc.gpsimd.ap_gather(gk[:], tab_sb[:], wr[:], channels=128,
                            num_elems=NE, d=3, num_idxs=N_TILE // 8)
        gk16 = gk[:].bitcast(F16)       # [128, 2048, 3, 2]
        gpitch = list(gk16.ap[0])

        # --- transpose-back (96 fp16 transposes) + blend -----------------
        # rounds r = (w, h): X2[p, tl, (g,q,dzdy)] fp16 PSUM; 16 tl0 each
        tmp = blp.tile([128, 6, 2048], F16, tag="tmp")
        l1 = blp.tile([128, 6, 1024], F16, tag="l1")
        for w in range(3):
            for h in range(2):
                r = w * 2 + h
                x2 = ps_x2.tile([128, 16, 128], F16, tag="x2")
                for tl0 in range(16):
                    stat = bass.AP(tensor=gk16.tensor,
                                   offset=gk16.offset + tl0 * 6 + w * 2 + h,
                                   ap=[gpitch, [96, 128]])
                    nc.tensor.transpose(x2[:, tl0, :], stat, ident16[:])
                # tmp[r] = X2 * V16   (enumer tl, g, (q,dzdy))
                nc.vector.tensor_tensor(
                    _ap(tmp, [[128, 16], [16, 8], [1, 16]], offset=r * 2048),
                    _ap(x2, [[128, 16], [16, 8], [1, 16]]),
                    _ap(v16, [[16, 16], [256, 8], [1, 16]]),
                    Alu.mult)


        # trees over (dz, dy, q): L1 sums dz-pairs, L2 dy, L3 q-pairs, L4 q.
        # tmp free = (r 6, tlg 128, dz 2, dy 2, q 4); (tl,g) merge to stride 16.
        nc.vector.tensor_tensor(
            _ap(l1, [[1024, 6], [8, 128], [1, 8]]),
            _ap(tmp, [[2048, 6], [16, 128], [1, 8]]),
            _ap(tmp, [[2048, 6], [16, 128], [1, 8]], offset=8),
            Alu.add)
        l2 = blp.tile([128, 6, 512], F16, tag="l2")
        nc.vector.tensor_tensor(
            _ap(l2, [[512, 6], [4, 128], [1, 4]]),
            _ap(l1, [[1024, 6], [8, 128], [1, 4]]),
            _ap(l1, [[1024, 6], [8, 128], [1, 4]], offset=4),
            Alu.add)
        l3 = blp.tile([128, 6, 256], F16, tag="l3")
        nc.vector.tensor_tensor(
            _ap(l3, [[256, 6], [2, 128], [1, 2]]),
            _ap(l2, [[512, 6], [4, 128], [1, 2]]),
            _ap(l2, [[512, 6], [4, 128], [1, 2]], offset=2),
            Alu.add)
        s = blp.tile([128, 6, 128], F16, tag="s")
        nc.vector.tensor_tensor(
            _ap(s, [[128, 6], [1, 128]]),
            _ap(l3, [[256, 6], [2, 128]]),
            _ap(l3, [[256, 6], [2, 128]], offset=1),
            Alu.add)

        # lerp: out_w = s[w,0] + wx * s[w,1]; out3[p, t, w]
        wxs = blp.tile([128, 3, 128], F16, tag="wxs")
        nc.vector.tensor_tensor(
            _ap(wxs, [[128, 3], [8, 16], [1, 8]]),
            _ap(s, [[256, 3], [8, 16], [1, 8]], offset=128),
            _ap(wx16, [[0, 3], [1, 16], [16, 8]]),
            Alu.mult)
        out3 = outp.tile([128, T, 3], F32, tag="out3")
        nc.vector.tensor_tensor(
            _ap(out3, [[1, 3], [3, 16], [48, 8]]),
            _ap(s, [[256, 3], [8, 16], [1, 8]]),
            _ap(wxs, [[128, 3], [8, 16], [1, 8]]),
            Alu.add)

        nc.sync.dma_start(
            out=bass.AP(tensor=out.tensor, offset=out.offset + j0 * 3,
                        ap=[[T * 3, 128], [1, T * 3]]),
            in_=out3[:])
    ctx.close()


def _pack_tables(grids):
    """grids: (4, 12, 8, 16, 16) -> per-view packed table [128, NE, 3] int32."""
    tabs = []
    for v in range(4):
        g = grids[v].reshape(12, NCELL).astype(np.float32)
        gp = np.zeros((12, NE + 274), np.float32)
        gp[:, :NCELL] = g
        t16 = np.zeros((16, NE, 3), np.uint32)
        for dz in range(2):
            for dy in range(2):
                dd = dz * 256 + dy * 16
                for q in range(4):
                    u = q * 4 + dz * 2 + dy
                    for w in range(3):
                        ch = 4 * w + q
                        lo = gp[ch, dd:dd + NE]
                        hi = gp[ch, dd + 1:dd + 1 + NE] - lo
                        lo16 = lo.astype(np.float16).view(np.uint16).astype(np.uint32)
                        hi16 = hi.astype(np.float16).view(np.uint16).astype(np.uint32)
                        t16[u, :, w] = lo16 | (hi16 << 16)
        tabs.append(np.tile(t16, (8, 1, 1)).view(np.int32))
    return tabs


def _shards(grid_xy, rgb, grids):
    ident = np.eye(128, dtype=np.float32)
    tabs = _pack_tables(np.asarray(grids))
    maps = []
    for k in range(N_CORES):
        v, h = k // 2, k % 2
        gxy_s = grid_xy[v, 0, h * HH:(h + 1) * HH].reshape(-1, 2)
        rgb_s = rgb[v, 0, h * HH:(h + 1) * HH].reshape(-1, 3)
        pad = P_PAD - P_CORE
        gxy_s = np.concatenate([gxy_s, np.zeros((pad, 2), np.float32)])
        rgb_s = np.concatenate([rgb_s, np.zeros((pad, 3), np.float32)])
        maps.append({
            "gxy": np.ascontiguousarray(gxy_s),
            "rgb": np.ascontiguousarray(rgb_s),
            "tab": np.ascontiguousarray(tabs[v]),
            "ident": ident,
        })
    return maps


def kernel(grid_xy, rgb, grids):
    if "nc" not in _cache:
        _cache["nc"] = _build(N_TILES)
    nc = _cache["nc"]
    maps = _shards(grid_xy, rgb, grids)
    res = bass_utils.run_bass_kernel_spmd(nc, maps, core_ids=list(range(N_CORES)))
    outv = np.empty((4, 1, H, W, 3), np.float32)
    for k in range(N_CORES):
        v, h = k // 2, k % 2
        o = res.results[k]["out"][:P_CORE].reshape(HH, W, 3)
        outv[v, 0, h * HH:(h + 1) * HH] = o
    return outv
